# revision 3
# baseline (speedup 1.0000x reference)
"""Bidirectional self-attention (B=4, T=2048, C=2048, H=16) on 8 Trainium2 cores.

Sharding: core c -> batch c//2, head-group c%2 (8 heads each). Each core runs
QKV projection (fp16 matmuls), RoPE, scores+softmax (exp on ScalarE, bf16 P),
P@V, and its slice of the output projection, producing a partial out^T
[C, T] fp32. Host sums the two head-group partials per batch and transposes.

Self-contained: hardcodes shapes and sharding; builds/caches one SPMD Bass
program and runs it on cores 0-7 via run_bass_kernel_spmd.
"""

import sys

if "/opt/trn_rl_repo" not in sys.path:
    sys.path.insert(0, "/opt/trn_rl_repo")

import numpy as np

import concourse.bass as bass
import concourse.tile as tile
from concourse import mybir
from concourse.bass import ts
from concourse.bass_utils import run_bass_kernel_spmd

B, T, C = 4, 2048, 2048
H = 16
D = 128
HPC = 8  # heads per core
N_CORES = 8
KC = C // 128  # 16 contraction chunks
TCH = 4  # t chunks of 512
ROPE_BASE = 10000.0
EXP_BIAS = -10.0
SM_SCALE = float(1.0 / np.sqrt(D))

F16 = mybir.dt.float16
BF16 = mybir.dt.bfloat16
F32 = mybir.dt.float32


def _legalize_waits(nc, cap=1):
    """This walrus build accepts only ONE semaphore wait (and update) per
    instruction. Split extras onto same-engine InstNoOps."""
    import bass_rust

    for fn in nc.m.functions:
        for blk in fn.blocks:
            out = []
            for ins in blk.instructions:
                si = ins.sync_info
                if si is None:
                    out.append(ins)
                    continue
                waits = list(si.on_wait or [])
                updates = list(si.on_update or [])
                new_waits, new_updates = waits, updates
                if len(waits) > cap:
                    keep = waits[-cap:]
                    extra = waits[:-cap]
                    for i in range(0, len(extra), cap):
                        out.append(
                            mybir.InstNoOp(
                                name=f"{ins.name}-wn{i}",
                                engine=ins.engine,
                                sync_info=bass_rust.SyncInfo(
                                    on_wait=extra[i : i + cap], on_update=[]
                                ),
                            )
                        )
                    new_waits = keep
                trailing = []
                if len(updates) > cap:
                    keep_u = updates[:cap]
                    extra_u = updates[cap:]
                    for i in range(0, len(extra_u), cap):
                        trailing.append(
                            mybir.InstNoOp(
                                name=f"{ins.name}-un{i}",
                                engine=ins.engine,
                                sync_info=bass_rust.SyncInfo(
                                    on_wait=[], on_update=extra_u[i : i + cap]
                                ),
                            )
                        )
                    new_updates = keep_u
                if new_waits is not waits or new_updates is not updates:
                    ins.sync_info = bass_rust.SyncInfo(
                        on_wait=new_waits, on_update=new_updates
                    )
                out.append(ins)
                out.extend(trailing)
            blk.instructions = out
    return nc


def build_nc():
    nc = bass.Bass("TRN2", target_bir_lowering=False, debug=False, num_devices=N_CORES)

    xT = nc.dram_tensor("xT", [C, T], F16, kind="ExternalInput").ap()
    wqkT = nc.dram_tensor("wqkT", [C, 2 * HPC * D], F16, kind="ExternalInput").ap()
    wvT = nc.dram_tensor("wvT", [C, HPC * D], F16, kind="ExternalInput").ap()
    wpT = nc.dram_tensor("wpT", [HPC * D, C], F16, kind="ExternalInput").ap()
    cosT = nc.dram_tensor("cosT", [D, T], F16, kind="ExternalInput").ap()
    sinT = nc.dram_tensor("sinT", [D, T], F16, kind="ExternalInput").ap()
    outT = nc.dram_tensor("outT", [C, T], F32, kind="ExternalOutput").ap()

    xT_r = xT.rearrange("(kc p) t -> p kc t", p=128)  # [128, 16, 2048]
    wqkT_r = wqkT.rearrange("(kc p) f -> p kc f", p=128)
    wvT_r = wvT.rearrange("(kc p) d -> p kc d", p=128)
    wpT_r = wpT.rearrange("(cc p) o -> p cc o", p=128)  # [128, 8, 2048]
    outT_r = outT.rearrange("(oi p) t -> p oi t", p=128)  # [128, 16, 2048]

    with tile.TileContext(nc) as tc:
        with (
            tc.tile_pool(name="const", bufs=1) as const,
            tc.tile_pool(name="perm", bufs=1) as perm,
        ):
            cos_sb = const.tile([D, T], F16, tag="cos")
            sin_sb = const.tile([D, T], F16, tag="sin")
            nc.sync.dma_start(cos_sb, cosT)
            nc.sync.dma_start(sin_sb, sinT)
            bias_sb = const.tile([128, 1], F32, tag="bias")
            nc.vector.memset(bias_sb, EXP_BIAS)
            ones_sb = const.tile([128, 128], BF16, tag="ones")
            nc.vector.memset(ones_sb, 1.0)

            qkT_sb = perm.tile([128, 2 * HPC, T], F16, tag="qkT")  # 8MB
            v_sb = perm.tile([128, KC, HPC * D], BF16, tag="v")  # 4MB

            # ---- Phase A: QKV projections + RoPE ----
            with (
                tc.tile_pool(name="astream", bufs=2) as astream,
                tc.tile_pool(name="aepi", bufs=3) as aepi,
                tc.tile_pool(name="aps", bufs=3, space="PSUM") as aps,
            ):
                # A1: q,k in [f, t] layout + RoPE
                for tch in range(TCH):
                    xt = astream.tile([128, KC, 512], F16, tag="xt")
                    nc.sync.dma_start(xt, xT_r[:, :, ts(tch, 512)])
                    cos_c = cos_sb[:, ts(tch, 512)]
                    sin_c = sin_sb[:, ts(tch, 512)]
                    for f2 in range(HPC):  # pairs of f-tiles
                        wf = astream.tile([128, KC, 256], F16, tag="wf")
                        nc.sync.dma_start(wf, wqkT_r[:, :, ts(f2, 256)])
                        for fi in range(2):
                            f = f2 * 2 + fi
                            ps = aps.tile([128, 512], F32, tag="qkps")
                            for kc in range(KC):
                                nc.tensor.matmul(
                                    ps,
                                    wf[:, kc, ts(fi, 128)],
                                    xt[:, kc],
                                    start=(kc == 0),
                                    stop=(kc == KC - 1),
                                )
                            qraw = aepi.tile([128, 512], F16, tag="qraw")
                            nc.scalar.copy(qraw, ps)
                            qrot = aepi.tile([128, 512], F16, tag="qrot")
                            nc.sync.dma_start(qrot[0:64, :], qraw[64:128, :])
                            nc.sync.dma_start(qrot[64:128, :], qraw[0:64, :])
                            dst = qkT_sb[:, f, ts(tch, 512)]
                            tmp = aepi.tile([128, 512], F16, tag="tmp")
                            nc.vector.tensor_mul(dst, qraw, cos_c)
                            nc.vector.tensor_mul(tmp, qrot, sin_c)
                            nc.vector.tensor_add(dst, dst, tmp)

                # A2: v in [t, dv] layout (bf16)
                with tc.tile_pool(name="wvp", bufs=1) as wvp:
                    wv = wvp.tile([128, KC, HPC * D], F16, tag="wv")
                    nc.sync.dma_start(wv, wvT_r)
                    for tch in range(TCH):
                        xt = astream.tile([128, KC, 512], F16, tag="xt")
                        nc.sync.dma_start(xt, xT_r[:, :, ts(tch, 512)])
                        for t128 in range(4):
                            stile = tch * 4 + t128
                            for dv in range(2):
                                vps = aps.tile([128, 512], F32, tag="vps")
                                for kc in range(KC):
                                    nc.tensor.matmul(
                                        vps,
                                        xt[:, kc, ts(t128, 128)],
                                        wv[:, kc, ts(dv, 512)],
                                        start=(kc == 0),
                                        stop=(kc == KC - 1),
                                    )
                                nc.vector.tensor_copy(
                                    v_sb[:, stile, ts(dv, 512)], vps
                                )

            # ---- Phases B (attention) and C (output projection) ----
            with tc.tile_pool(name="bc", bufs=1) as bc:
                wp = bc.tile([128, HPC, T], F16, tag="wp")
                nc.sync.dma_start(wp, wpT_r)
                yT_sb = bc.tile([128, HPC, T], F16, tag="yT")  # 4MB

                with (
                    tc.tile_pool(name="pb", bufs=2) as pb,
                    tc.tile_pool(name="psb", bufs=2, space="PSUM") as psb,
                ):
                    for h in range(HPC):
                        qh = qkT_sb[:, h]
                        kh = qkT_sb[:, HPC + h]
                        for tch in range(TCH):
                            q_c = qh[:, ts(tch, 512)]
                            yt_ps = psb.tile([128, 512], F32, tag="yt")
                            den_ps = psb.tile([128, 512], F32, tag="den")
                            for half in range(2):
                                ph = pb.tile([128, 8, 512], BF16, tag="p")
                                for g in range(4):
                                    st0 = half * 8 + g * 2
                                    sc = psb.tile([128, 1024], F32, tag="sc")
                                    nc.tensor.matmul(
                                        sc[:, 0:512], kh[:, ts(st0, 128)], q_c
                                    )
                                    nc.tensor.matmul(
                                        sc[:, 512:1024], kh[:, ts(st0 + 1, 128)], q_c
                                    )
                                    nc.scalar.activation(
                                        ph[:, g * 2 : g * 2 + 2],
                                        sc,
                                        mybir.ActivationFunctionType.Exp,
                                        bias=bias_sb[:],
                                        scale=SM_SCALE,
                                    )
                                for j in range(8):
                                    st = half * 8 + j
                                    nc.tensor.matmul(
                                        yt_ps,
                                        v_sb[:, st, ts(h, 128)],
                                        ph[:, j],
                                        start=(st == 0),
                                        stop=(st == 15),
                                    )
                                    nc.tensor.matmul(
                                        den_ps,
                                        ones_sb,
                                        ph[:, j],
                                        start=(st == 0),
                                        stop=(st == 15),
                                    )
                            rec = pb.tile([128, 512], F32, tag="rec")
                            nc.vector.reciprocal(rec, den_ps)
                            nc.vector.tensor_mul(
                                yT_sb[:, h, ts(tch, 512)], yt_ps, rec
                            )

                with (
                    tc.tile_pool(name="pc", bufs=2) as pc,
                    tc.tile_pool(name="pcps", bufs=2, space="PSUM") as pcps,
                ):
                    for oi in range(16):
                        stage = pc.tile([128, TCH, 512], F32, tag="stage")
                        for tch in range(TCH):
                            ops = pcps.tile([128, 512], F32, tag="o")
                            for cc in range(HPC):
                                nc.tensor.matmul(
                                    ops,
                                    wp[:, cc, ts(oi, 128)],
                                    yT_sb[:, cc, ts(tch, 512)],
                                    start=(cc == 0),
                                    stop=(cc == HPC - 1),
                                )
                            nc.vector.tensor_copy(stage[:, tch], ops)
                        nc.sync.dma_start(outT_r[:, oi], stage)

    _legalize_waits(nc)
    return nc


_NC_CACHE = []


def _get_nc():
    if not _NC_CACHE:
        _NC_CACHE.append(build_nc())
    return _NC_CACHE[0]


def _rope_tables():
    inv = 1.0 / (ROPE_BASE ** (np.arange(0, D, 2, dtype=np.float64) / D))  # [64]
    t = np.arange(T, dtype=np.float64)
    freqs = np.outer(t, inv)  # [T, 64]
    emb = np.concatenate([freqs, freqs], axis=1)  # [T, 128]
    cosT = np.cos(emb).T.astype(np.float16)  # [128, T]
    sinT = np.sin(emb).T.astype(np.float64)
    sign = np.where(np.arange(D) < D // 2, -1.0, 1.0)[:, None]
    sinT = (sinT * sign).astype(np.float16)
    return np.ascontiguousarray(cosT), np.ascontiguousarray(sinT)


def kernel(x, w_attn, w_proj):
    x = np.asarray(x, dtype=np.float32)
    w_attn = np.asarray(w_attn, dtype=np.float32)
    w_proj = np.asarray(w_proj, dtype=np.float32)

    nc = _get_nc()
    cosT, sinT = _rope_tables()

    in_maps = []
    for c in range(N_CORES):
        b, hg = divmod(c, 2)
        rows = slice(hg * 1024, hg * 1024 + 1024)
        q_w = w_attn[0 * C:][rows.start : rows.stop]
        k_w = w_attn[1 * C:][rows.start : rows.stop]
        v_w = w_attn[2 * C:][rows.start : rows.stop]
        wqkT = np.ascontiguousarray(
            np.concatenate([q_w, k_w], axis=0).T.astype(np.float16)
        )
        wvT = np.ascontiguousarray(v_w.T.astype(np.float16))
        wpT = np.ascontiguousarray(w_proj[:, rows].T.astype(np.float16))
        xT = np.ascontiguousarray(x[b].T.astype(np.float16))
        in_maps.append(
            {
                "xT": xT,
                "wqkT": wqkT,
                "wvT": wvT,
                "wpT": wpT,
                "cosT": cosT,
                "sinT": sinT,
            }
        )

    res = run_bass_kernel_spmd(nc, in_maps, list(range(N_CORES)))
    out = np.empty((B, T, C), dtype=np.float32)
    for b in range(B):
        acc = res.results[2 * b]["outT"] + res.results[2 * b + 1]["outT"]
        out[b] = acc.T
    return out
